# revision 21
# baseline (speedup 1.0000x reference)
"""Trainium2 Bass kernel for nn_AttentionCombinerConv.

Strategy (pure data parallel, batch sharded 8 ways):
  Per core (8 batch elements, 4096 positions, 64 img + 16 basis channels):
    scores = feats @ U_b where U_b = Wk_head-blocks @ q_head  (collapses the
    [262k,80]@[80,64] K-projection 16x), softmax over positions, then
    pooled = (attn^T @ feats) @ Wv-blocks + bv  (collapses the V-projection).
  Scores run in fp32 with positions on PSUM partitions; pooling runs in bf16
  (attention weights in [0,1], values accumulate in fp32 PSUM).
"""

import numpy as np
import ml_dtypes
from contextlib import ExitStack

import concourse.bass as bass
import concourse.tile as tile
from concourse import bacc, mybir
from concourse import bass_utils

BF16 = ml_dtypes.bfloat16
F32 = np.float32

B, H, W, C = 64, 64, 64, 64
HW = H * W            # 4096 positions
NH, DPH = 4, 16       # heads, depth per head
CF = 64               # conv filters
BD = 16               # basis dim
FIN = C + BD          # 80
KA = FIN + 1          # 81 (ones row carries bk·q term)
NCORES = 8
BL = B // NCORES      # 8 batch elements per core
NCH = HW // 128       # 32 position chunks of 128

_cache = {}


def _build():
    dt = mybir.dt
    nc = bacc.Bacc(
        "TRN2",
        target_bir_lowering=False,
        debug=False,
        enable_asserts=False,
        num_devices=NCORES,
    )

    # ---- DRAM I/O (per core) ----
    featst_d = nc.dram_tensor("featst", [BL, KA, HW], dt.float32, kind="ExternalInput").ap()
    imgbf_d = nc.dram_tensor("imgbf", [BL, 128, NCH * C], dt.bfloat16, kind="ExternalInput").ap()
    basisbf_d = nc.dram_tensor("basisbf", [128, NCH * BD], dt.bfloat16, kind="ExternalInput").ap()
    wqsp_d = nc.dram_tensor("wqsp", [128, 1024], dt.float32, kind="ExternalInput").ap()
    states_d = nc.dram_tensor("states", [128, 64], dt.float32, kind="ExternalInput").ap()
    wktb_d = nc.dram_tensor("wktb", [128, KA], dt.float32, kind="ExternalInput").ap()
    wv_d = nc.dram_tensor("wv", [FIN, CF], dt.float32, kind="ExternalInput").ap()
    bq_d = nc.dram_tensor("bq", [128, 1], dt.float32, kind="ExternalInput").ap()
    bv_d = nc.dram_tensor("bv", [DPH, NH], dt.float32, kind="ExternalInput").ap()
    onesc_d = nc.dram_tensor("onesc", [128, 1], dt.float32, kind="ExternalInput").ap()
    onesr_d = nc.dram_tensor("onesr", [1, 128], dt.float32, kind="ExternalInput").ap()
    i32f_d = nc.dram_tensor("i32f", [32, 32], dt.float32, kind="ExternalInput").ap()
    i128f_d = nc.dram_tensor("i128f", [128, 128], dt.float32, kind="ExternalInput").ap()

    pooledt_d = nc.dram_tensor("pooledt", [CF, BL], dt.float32, kind="ExternalOutput").ap()
    meanat_d = nc.dram_tensor("meanat", [BL, HW], dt.float32, kind="ExternalOutput").ap()

    with tile.TileContext(nc) as tc, ExitStack() as ctx:
        cp = ctx.enter_context(tc.tile_pool(name="consts", bufs=1))
        ftp = ctx.enter_context(tc.tile_pool(name="ft", bufs=8))
        imp = ctx.enter_context(tc.tile_pool(name="im", bufs=1))
        wk = ctx.enter_context(tc.tile_pool(name="wk", bufs=1))
        mnp = ctx.enter_context(tc.tile_pool(name="mn", bufs=2))
        pscr = ctx.enter_context(tc.tile_pool(name="pscr", bufs=2, space="PSUM"))
        ppf = ctx.enter_context(tc.tile_pool(name="ppf", bufs=2, space="PSUM"))
        psm = ctx.enter_context(tc.tile_pool(name="psm", bufs=2, space="PSUM"))

        AX = mybir.AxisListType
        OP = mybir.AluOpType
        AF = mybir.ActivationFunctionType

        # ---- load constants ----
        wqsps = cp.tile([128, 1024], dt.float32, tag="wqsps")
        nc.sync.dma_start(wqsps[:], wqsp_d)
        states = cp.tile([128, 64], dt.float32, tag="states")
        nc.sync.dma_start(states[:], states_d)
        wktbs = cp.tile([128, KA], dt.float32, tag="wktbs")
        nc.sync.dma_start(wktbs[:], wktb_d)
        wvs = cp.tile([FIN, CF], dt.float32, tag="wvs")
        nc.sync.dma_start(wvs[:], wv_d)
        bqs = cp.tile([128, 1], dt.float32, tag="bqs")
        nc.sync.dma_start(bqs[:], bq_d)
        bv4s = cp.tile([DPH, NH], dt.float32, tag="bv4s")
        nc.sync.dma_start(bv4s[:], bv_d)
        onescs = cp.tile([128, 1], dt.float32, tag="onescs")
        nc.sync.dma_start(onescs[:], onesc_d)
        onesrs = cp.tile([1, 128], dt.float32, tag="onesrs")
        nc.sync.dma_start(onesrs[:], onesr_d)
        i32fs = cp.tile([32, 32], dt.float32, tag="i32fs")
        nc.sync.dma_start(i32fs[:], i32f_d)
        i128fs = cp.tile([128, 128], dt.float32, tag="i128fs")
        nc.sync.dma_start(i128fs[:], i128f_d)
        basisbfs = cp.tile([128, NCH * BD], dt.bfloat16, tag="basisbfs")
        nc.sync.dma_start(basisbfs[:], basisbf_d)

        # ---- big streaming inputs ----
        imgs = imp.tile([128, BL * NCH * C], dt.bfloat16, tag="imgs")
        for b in range(BL):
            nc.sync.dma_start(imgs[:, b * NCH * C:(b + 1) * NCH * C], imgbf_d[b])
        fts = []
        for b in range(BL):
            ft = ftp.tile([KA, HW], dt.float32, tag="ft")
            nc.sync.dma_start(ft[:], featst_d[b])
            fts.append(ft)

        # ---- q = state @ Wq + bq  (head-blocks padded to 32 rows) ----
        qp = psm.tile([128, BL], dt.float32, tag="sm")
        for k in range(8):
            nc.tensor.matmul(
                qp[:], wqsps[:, k * 128:(k + 1) * 128], states[:, k * BL:(k + 1) * BL],
                start=(k == 0), stop=(k == 7),
            )
        qs32 = wk.tile([128, BL], dt.float32, tag="qs32")
        nc.vector.tensor_scalar_add(qs32[:], qp[:], bqs[:, 0:1])

        # ---- U[ch_aug, (n,b)] = WkT-blocks @ q-blocks ----
        # One base-0 matmul: rhs is q replicated into per-head masked column
        # blocks (rows outside head n's 32-row block are zero).
        qmask = wk.tile([128, NH * BL], dt.float32, tag="qmask")
        nc.vector.memset(qmask[:], 0.0)
        for n in range(NH):
            nc.vector.tensor_copy(
                qmask[32 * n:32 * n + 32, n * BL:(n + 1) * BL],
                qs32[32 * n:32 * n + 32, :],
            )
        up = psm.tile([KA, NH * BL], dt.float32, tag="sm")
        nc.tensor.matmul(up[:], wktbs[:], qmask[:], start=True, stop=True)
        usb = wk.tile([KA, NH * BL], dt.float32, tag="usb")  # cols b-major: 4b+n
        nc.vector.tensor_copy(
            usb[:].rearrange("p (b n) -> p n b", n=NH),
            up[:].rearrange("p (n b) -> p n b", b=BL),
        )

        # ---- scores (pos on partitions) + exp ----
        enat = wk.tile([128, BL * 128], dt.float32, tag="enat")  # cols (b, c, n)
        for b in range(BL):
            sp = pscr.tile([128, 128], dt.float32, tag="sc")
            for c in range(NCH):
                nc.tensor.matmul(
                    sp[:, c * NH:(c + 1) * NH],
                    fts[b][:, c * 128:(c + 1) * 128],
                    usb[:, b * NH:(b + 1) * NH],
                    start=True, stop=True,
                )
            nc.scalar.activation(enat[:, b * 128:(b + 1) * 128], sp[:], AF.Exp)

        # ---- softmax sums over positions via ones-matmul ----
        partials = wk.tile([1, BL * 128], dt.float32, tag="partials")
        for b in range(BL):
            smp = psm.tile([1, 128], dt.float32, tag="sm")
            nc.tensor.matmul(
                smp[:], onescs[:], enat[:, b * 128:(b + 1) * 128],
                start=True, stop=True,
            )
            nc.scalar.copy(partials[:, b * 128:(b + 1) * 128], smp[:])
        sums = wk.tile([1, BL * NH], dt.float32, tag="sums")
        nc.vector.tensor_reduce(
            sums[:].rearrange("p (b n) -> p b n", n=NH),
            partials[:].rearrange("p (b c n) -> p b n c", c=NCH, n=NH),
            axis=AX.X, op=OP.add,
        )
        rrow = wk.tile([1, BL * NH], dt.float32, tag="rrow")
        nc.vector.reciprocal(rrow[:], sums[:])
        # broadcast r down all 128 partitions: ones-col outer product
        rtp = psm.tile([128, BL * NH], dt.float32, tag="sm")
        nc.tensor.matmul(rtp[:], onesrs[:], rrow[:], start=True, stop=True)
        rnat = wk.tile([128, BL * NH], dt.float32, tag="rnat")
        nc.vector.tensor_copy(rnat[:], rtp[:])

        # ---- attn = e * (1/sum), cast to bf16 ----
        attnbf = wk.tile([128, BL * 128], dt.bfloat16, tag="attnbf")
        for b in range(BL):
            for n in range(NH):
                col = b * NH + n
                nc.vector.tensor_scalar(
                    attnbf[:, b * 128 + n:(b + 1) * 128:NH],
                    enat[:, b * 128 + n:(b + 1) * 128:NH],
                    rnat[:, col:col + 1], None, OP.mult,
                )

        # ---- mean over heads -> [8, 4096] ----
        for b in range(BL):
            mr = wk.tile([128, NCH], dt.float32, tag="mr", bufs=2)
            nc.vector.tensor_reduce(
                mr[:],
                attnbf[:, b * 128:(b + 1) * 128].rearrange("p (c n) -> p c n", n=NH),
                axis=AX.X, op=OP.add,
            )
            mtp = psm.tile([32, 128], dt.float32, tag="sm")
            nc.tensor.transpose(mtp[:], mr[:], i128fs[:])
            msb = mnp.tile([32, 128], dt.float32, tag="msb")
            nc.vector.tensor_scalar(msb[:], mtp[:], 0.25, None, OP.mult)
            nc.sync.dma_start(meanat_d[b].rearrange("(c p) -> c p", p=128), msb[:])

        # ---- pooled: PF[(b,n), ch] = attn^T @ feats ----
        pftp = psm.tile([FIN, NH * BL], dt.float32, tag="pft", bufs=1)
        for b in range(BL):
            pfi = ppf.tile([NH, C], dt.float32, tag="pfi")
            pfba = ppf.tile([NH, BD], dt.float32, tag="pfi")
            for c in range(NCH):
                at = attnbf[:, b * 128 + c * NH: b * 128 + (c + 1) * NH]
                nc.tensor.matmul(
                    pfi[:], at, imgs[:, b * NCH * C + c * C: b * NCH * C + (c + 1) * C],
                    start=(c == 0), stop=(c == NCH - 1),
                )
                nc.tensor.matmul(
                    pfba[:], at, basisbfs[:, c * BD:(c + 1) * BD],
                    start=(c == 0), stop=(c == NCH - 1),
                )
            pfb = wk.tile([NH, FIN], dt.float32, tag="pfb", bufs=2)
            nc.scalar.copy(pfb[:, 0:C], pfi[:])
            nc.vector.tensor_copy(pfb[:, C:FIN], pfba[:])
            nc.tensor.transpose(
                pftp[:, NH * b:NH * (b + 1)], pfb[:], i32fs[0:NH, 0:NH]
            )

        # ---- pooledT = Wv-blocks @ PF^T + bv ----
        pfts = wk.tile([FIN, NH * BL], dt.float32, tag="pfts")
        nc.vector.tensor_copy(pfts[:], pftp[:])
        for n in range(NH):
            fin = psm.tile([DPH, BL], dt.float32, tag="sm")
            nc.tensor.matmul(
                fin[:],
                wvs[:, DPH * n:DPH * (n + 1)],
                pfts[:].rearrange("p (b n) -> p n b", n=NH)[:, n, :],
                start=True, stop=True,
            )
            pout = wk.tile([DPH, BL], dt.float32, tag="pout", bufs=2)
            nc.vector.tensor_scalar_add(pout[:], fin[:], bv4s[:, n:n + 1])
            nc.sync.dma_start(pooledt_d[DPH * n:DPH * (n + 1), :], pout[:])

    nc.compile()
    return nc


def _prep_inputs(image, state1, state2, Wq, bq, Wk, bk, Wv, bv, spatial_basis):
    """Host-side layout prep; returns per-core in_maps."""
    img = np.ascontiguousarray(image.reshape(B, HW, C), dtype=F32)
    basis = np.ascontiguousarray(spatial_basis.reshape(HW, BD), dtype=F32)
    basisT = basis.T.copy()  # [16, 4096]
    ones_row = np.ones((1, HW), F32)

    # basis tiles for pooling: [128, (c, ch)] bf16
    basisbf = np.ascontiguousarray(
        basis.reshape(NCH, 128, BD).transpose(1, 0, 2).reshape(128, NCH * BD)
    ).astype(BF16)

    state = np.concatenate([state1, state2], axis=1).astype(F32)  # [64, 1024]

    # Wq with head-blocks padded to 32 output rows: col 32n+j = Wq[:, 16n+j]
    wqp = np.zeros((1024, 128), F32)
    for n in range(NH):
        wqp[:, 32 * n:32 * n + DPH] = Wq[:, DPH * n:DPH * (n + 1)]
    wqsp = np.ascontiguousarray(
        wqp.reshape(8, 128, 128).transpose(1, 0, 2).reshape(128, 1024)
    )
    bq32 = np.zeros((128, 1), F32)
    for n in range(NH):
        bq32[32 * n:32 * n + DPH, 0] = bq[DPH * n:DPH * (n + 1)]

    # WkT augmented with bk column, head-blocks padded to 32 rows
    wktb = np.zeros((128, KA), F32)
    for n in range(NH):
        wktb[32 * n:32 * n + DPH, :FIN] = Wk.T[DPH * n:DPH * (n + 1), :]
        wktb[32 * n:32 * n + DPH, FIN] = bk[DPH * n:DPH * (n + 1)]

    consts = {
        "basisbf": basisbf,
        "wqsp": wqsp,
        "wktb": wktb,
        "wv": np.ascontiguousarray(Wv, dtype=F32),
        "bq": bq32,
        "bv": np.ascontiguousarray(np.asarray(bv, F32).reshape(NH, DPH).T),
        "onesc": np.ones((128, 1), F32),
        "onesr": np.ones((1, 128), F32),
        "i32f": np.eye(32, dtype=F32),
        "i128f": np.eye(128, dtype=F32),
    }

    in_maps = []
    for r in range(NCORES):
        sl = slice(BL * r, BL * (r + 1))
        featst = np.empty((BL, KA, HW), F32)
        for i, b in enumerate(range(BL * r, BL * (r + 1))):
            featst[i, :C] = img[b].T
            featst[i, C:FIN] = basisT
            featst[i, FIN] = ones_row
        imgbf = (
            img[sl].reshape(BL, NCH, 128, C).transpose(0, 2, 1, 3).reshape(BL, 128, NCH * C)
        ).astype(BF16)
        st = state[sl].T  # [1024, 8]
        states = np.ascontiguousarray(
            st.reshape(8, 128, BL).transpose(1, 0, 2).reshape(128, 8 * BL)
        )
        m = {"featst": featst, "imgbf": imgbf, "states": states}
        m.update(consts)
        in_maps.append(m)
    return in_maps


def _get_prog():
    if "nc" not in _cache:
        _cache["nc"] = _build()
    return _cache["nc"]


def run_cores(in_maps, trace=False):
    nc = _get_prog()
    return bass_utils.run_bass_kernel_spmd(
        nc, in_maps, core_ids=list(range(NCORES)), trace=trace
    )


def kernel(image, state1, state2, extra, Wq, bq, Wk, bk, Wv, bv, spatial_basis):
    image = np.asarray(image, F32)
    in_maps = _prep_inputs(
        image,
        np.asarray(state1, F32), np.asarray(state2, F32),
        np.asarray(Wq, F32), np.asarray(bq, F32),
        np.asarray(Wk, F32), np.asarray(bk, F32),
        np.asarray(Wv, F32), np.asarray(bv, F32),
        np.asarray(spatial_basis, F32),
    )
    res = run_cores(in_maps).results

    pooled = np.concatenate([np.asarray(res[r]["pooledt"]).T for r in range(NCORES)], axis=0)
    mean = np.concatenate([np.asarray(res[r]["meanat"]) for r in range(NCORES)], axis=0)
    out = np.concatenate([pooled, np.asarray(extra, F32)], axis=1).astype(F32)
    return out, mean.reshape(B, H, W).astype(F32)
